# revision 1
# baseline (speedup 1.0000x reference)
"""Trainium2 Bass kernel for nn_MessageUpdatePore (gnn_message_passing).

Algebraic collapse of the reference:
  Because idx2_oh == one_hot(idx2), the [B,E,F,K] one-hot expansion, the
  permutation-equivariant group-averaged linear, and the post-activation
  slot selection reduce to per-edge dense algebra:
      z_g   = concat(sites1[b][idx1], sites2[b][idx2], bonds[b]) @ W_eq[g]
      lat0  = sum_g c[g, idx2[e]]/G * z_g          (c==1 when perms1==perms2,
                                                    then W folds to mean_g W_eq)
      lat   = leaky_relu(lat0 + b_eq)
      lat  *= sigmoid(lat @ W_att + b_att)
      out[b, idx2[e]] += lat                        (scatter-add over edges)
  The site-feature contributions fold host-side into per-node tables
  A1 = sites1 @ W[:CIN], A2 = sites2 @ W[CIN:2CIN] (O(nodes) preprocessing);
  the per-edge device work is three one-hot/bond matmuls accumulated in PSUM
  (both batches side by side in one PSUM tile), the activation pipeline, and
  a one-hot scatter matmul. The edge dim E is sharded across 8 cores and the
  [B,K,O] partials are summed on the host.
"""

from contextlib import ExitStack

import numpy as np

import concourse.bacc as bacc
import concourse.mybir as mybir
import concourse.tile as tile
from concourse.bass_utils import run_bass_kernel_spmd

B, E, N1, K, CIN, CB, COUT, G = 2, 2048, 96, 32, 64, 32, 64, 4
F = 2 * CIN + CB           # 160
NCORES = 8
ES = E // NCORES           # 256 edges per core
ECH = ES // 128            # 2 edge chunks of 128
NEG_SLOPE = 0.01
f32 = mybir.dt.float32

_programs: dict = {}

# feature toggles (module-level so probes can flip them before build)
WARMUP = 0          # number of PE warm-up dummy matmuls (0 = off; measured no-op:
                    # 213ns/64-col fp32 MM is the streaming rate, not a cold clock)
D128_SCALAR = True  # issue d128 DMA on the scalar HWDGE ring
ATT_V3 = True       # fused attention: mul + 3D reduce + single sigmoid


def _layouts(NG: int, use_beq: bool):
    """Column layouts of the three partition-height-grouped input tensors."""
    NO = NG * COUT
    off = {}
    # d128 [128, x128]
    off["oh2"] = 0                       # ECH chunks of [128, K]
    off["wattc"] = ECH * K               # [128, B*COUT] (W_att row, tiled per batch)
    off["batt"] = off["wattc"] + B * COUT  # [128, 1]
    off["coeff"] = off["batt"] + 1       # ECH chunks of [128, NG]
    off["beq"] = off["coeff"] + ECH * NG
    off["x128"] = off["beq"] + (COUT if use_beq else 0)
    # d96 [96, x96]: critical-first ordering — chunk-0 one-hot + A1 tables in
    # the front block (first DMA), chunk-1 one-hot behind (second DMA)
    off["oh1T0"] = 0                     # [96, 128]
    off["A1"] = 128                      # B blocks of [96, NO]
    off["oh1T1"] = 128 + B * NO          # [96, 128]
    off["x96"] = off["oh1T1"] + 128
    # d64 [64, x64]: per batch, a contraction-stacked pair so one matmul
    # computes gather2 + bonds@W3: lhsT rows 0:32 = oh2T, rows 32:64 = bondsT;
    # rhs rows 0:32 = A2[b], rows 32:64 = W3. (Matmul operands must sit at
    # the same base partition on HW — non-zero bases fault the exec unit, so
    # batches are column blocks at base 0, not partition-row blocks.)
    off["combo"] = 0                     # B blocks of [64, ES + NO]
    off["x64"] = B * (ES + NO)
    return off


def _build_program(NG: int, use_beq: bool):
    NO = NG * COUT
    off = _layouts(NG, use_beq)
    mult, add = mybir.AluOpType.mult, mybir.AluOpType.add

    nc = bacc.Bacc(
        "TRN2", target_bir_lowering=False, debug=False, num_devices=NCORES
    )
    d128 = nc.dram_tensor("d128", [128, off["x128"]], f32, kind="ExternalInput")
    d96 = nc.dram_tensor("d96", [N1, off["x96"]], f32, kind="ExternalInput")
    d64 = nc.dram_tensor("d64", [64, off["x64"]], f32, kind="ExternalInput")
    out_d = nc.dram_tensor("out", [K, B * COUT], f32, kind="ExternalOutput")

    with tile.TileContext(nc) as tc, ExitStack() as ctx:
        const = ctx.enter_context(tc.tile_pool(name="const", bufs=1))
        work = ctx.enter_context(tc.tile_pool(name="work", bufs=2))
        ps_z = ctx.enter_context(tc.tile_pool(name="ps_z", bufs=2, space="PSUM"))
        ps_o = ctx.enter_context(tc.tile_pool(name="ps_o", bufs=1, space="PSUM"))
        ps_w = ctx.enter_context(tc.tile_pool(name="ps_w", bufs=1, space="PSUM"))

        # Warm the PE HAM clock gate during the DMA/preamble window: dummy
        # matmuls on a scratch tile with no input dependencies. Cold PE runs
        # at 1.2GHz; ~3.4us of activity unlocks 2.4GHz for the real matmuls.
        if WARMUP:
            wsrc = const.tile([128, 128], f32, tag="wsrc", name="wsrc")
            nc.vector.memset(wsrc[:], 1.0)
            warm = ps_w.tile([128, 128], f32)
            for _ in range(WARMUP):
                nc.tensor.matmul(warm[:], wsrc[:], wsrc[:], start=True, stop=True)

        # One input DMA per engine ring so all three stream in parallel and
        # each consumer waits only on its own tensor's completion. The first
        # z matmul needs only t96a (smaller => earlier completion).
        t96a = const.tile([N1, 128], f32, tag="t96a", name="t96a")
        nc.sync.dma_start(t96a[:], d96[:, 0:128])
        tA1 = const.tile([N1, B * NO], f32, tag="tA1", name="tA1")
        nc.gpsimd.dma_start(tA1[:], d96[:, off["A1"] : off["A1"] + B * NO])
        t96b = const.tile([N1, 128], f32, tag="t96b", name="t96b")
        nc.sync.dma_start(t96b[:], d96[:, off["oh1T1"] :])
        t64 = const.tile([64, off["x64"]], f32, tag="t64", name="t64")
        nc.scalar.dma_start(t64[:], d64[:])
        t128 = const.tile([128, off["x128"]], f32, tag="t128", name="t128")
        nc.gpsimd.dma_start(t128[:], d128[:])

        a1cat = tA1[:, :]                                    # [96, B*NO]
        wattc = t128[:, off["wattc"] : off["wattc"] + B * COUT]
        watt = t128[:, off["wattc"] : off["wattc"] + COUT]
        batt = t128[:, off["batt"] : off["batt"] + 1]

        # Emit both t96-gated gather matmuls first so the PE has work while
        # t64 (scalar ring) is still completing, then the combo matmuls.
        oh1 = [t96a[:, 0:128], t96b[:, 0:128]]
        zs = []
        for ec in range(ECH):
            z = ps_z.tile([128, B * NO], f32, tag="z", name=f"z{ec}")
            nc.tensor.matmul(z[:], oh1[ec], a1cat, start=True, stop=False)
            zs.append(z)
        for ec in range(ECH):
            for b in range(B):
                base = off["combo"] + b * (ES + NO)
                combo = t64[:, base + ec * 128 : base + (ec + 1) * 128]  # [64, 128]
                stack = t64[:, base + ES : base + ES + NO]               # [64, NO]
                nc.tensor.matmul(
                    zs[ec][:, b * NO : (b + 1) * NO], combo, stack,
                    start=False, stop=(b == B - 1),
                )

        latf = []
        for ec in range(ECH):
            z = zs[ec]
            lat_ec = const.tile(
                [128, B * COUT], f32, tag=f"latf{ec}", name=f"latf{ec}"
            )
            latf.append(lat_ec)

            if NG == 1:
                # leaky_relu(x) = max(x, NEG_SLOPE*x), both batches at once
                tmp = work.tile([128, B * COUT], f32, tag="tmp", name="tmp")
                nc.vector.tensor_scalar_mul(tmp[:], z[:], NEG_SLOPE)
                nc.vector.tensor_max(lat_ec[:], tmp[:], z[:])
            else:
                csl = t128[:, off["coeff"] + ec * NG : off["coeff"] + (ec + 1) * NG]
                for b in range(B):
                    zb = z[:, b * NO : (b + 1) * NO]
                    acc_sb = work.tile([128, COUT], f32, tag="acc0", name="acc0")
                    nc.vector.tensor_scalar_mul(acc_sb[:], zb[:, 0:COUT], csl[:, 0:1])
                    for g in range(1, NG):
                        nxt = work.tile(
                            [128, COUT], f32, tag=f"acc{g % 2}", name=f"acc{g % 2}"
                        )
                        nc.vector.scalar_tensor_tensor(
                            nxt[:], zb[:, g * COUT : (g + 1) * COUT],
                            csl[:, g : g + 1], acc_sb[:], op0=mult, op1=add,
                        )
                        acc_sb = nxt
                    acc = acc_sb[:]
                    if use_beq:
                        beq = t128[:, off["beq"] : off["beq"] + COUT]
                        accb = work.tile([128, COUT], f32, tag="accb", name="accb")
                        nc.vector.tensor_add(accb[:], acc, beq)
                        acc = accb[:]
                    tmp = work.tile([128, COUT], f32, tag="tmp", name="tmp")
                    nc.vector.tensor_scalar_mul(tmp[:], acc, NEG_SLOPE)
                    nc.vector.tensor_max(
                        lat_ec[:, b * COUT : (b + 1) * COUT], tmp[:], acc
                    )

            if ATT_V3:
                # attention gate: one dot via elementwise mul + 3D-view reduce,
                # one sigmoid for both batches, per-batch rescale
                junk = work.tile([128, B * COUT], f32, tag="junk", name="junk")
                nc.vector.tensor_mul(junk[:], lat_ec[:], wattc)
                s2 = work.tile([128, B], f32, tag="s2", name="s2")
                nc.vector.tensor_reduce(
                    out=s2[:], in_=junk[:].rearrange("p (b o) -> p b o", b=B),
                    axis=mybir.AxisListType.X, op=add,
                )
                att2 = work.tile([128, B], f32, tag="att2", name="att2")
                nc.scalar.activation(
                    att2[:], s2[:], mybir.ActivationFunctionType.Sigmoid, bias=batt
                )
                for b in range(B):
                    lat = lat_ec[:, b * COUT : (b + 1) * COUT]
                    nc.vector.tensor_scalar_mul(lat, lat, att2[:, b : b + 1])
            else:
                for b in range(B):
                    lat = lat_ec[:, b * COUT : (b + 1) * COUT]
                    junk = work.tile([128, COUT], f32, tag="junk", name="junk")
                    scol = work.tile([128, 1], f32, tag="scol", name="scol")
                    nc.vector.scalar_tensor_tensor(
                        out=junk[:], in0=lat, scalar=1.0, in1=watt,
                        op0=mult, op1=mult, accum_out=scol[:],
                    )
                    atc = work.tile([128, 1], f32, tag="atc", name="atc")
                    nc.scalar.activation(
                        atc[:], scol[:], mybir.ActivationFunctionType.Sigmoid,
                        bias=batt,
                    )
                    nc.vector.tensor_scalar_mul(lat, lat, atc[:])

        # scatter per (chunk, batch): each 64-col matmul only needs its own
        # half of latf, so it can fire as soon as that batch's scale lands
        o_ps = ps_o.tile([K, B * COUT], f32)
        for ec in range(ECH):
            oh2c = t128[:, off["oh2"] + ec * K : off["oh2"] + (ec + 1) * K]
            for b in range(B):
                nc.tensor.matmul(
                    o_ps[:, b * COUT : (b + 1) * COUT], oh2c,
                    latf[ec][:, b * COUT : (b + 1) * COUT],
                    start=(ec == 0 and b == 0), stop=(ec == ECH - 1 and b == B - 1),
                )
        o_sb = work.tile([K, B * COUT], f32, tag="osb", name="osb")
        nc.vector.tensor_copy(o_sb[:], o_ps[:])
        nc.sync.dma_start(out_d[:], o_sb[:])

    nc.compile()
    return nc


def _get_program(NG: int, use_beq: bool):
    key = (NG, use_beq)
    if key not in _programs:
        _programs[key] = _build_program(NG, use_beq)
    return _programs[key]


def _prepare(inputs):
    """Host-side preprocessing: group fold, node-table fold, one-hots, shards."""
    sites1 = np.ascontiguousarray(inputs["sites1"], np.float32)
    sites2 = np.ascontiguousarray(inputs["sites2"], np.float32)
    bonds = np.ascontiguousarray(inputs["bonds"], np.float32)
    W_eq = np.asarray(inputs["W_eq"], np.float32)
    b_eq = np.asarray(inputs["b_eq"], np.float32)
    W_att = np.asarray(inputs["W_att"], np.float32)
    b_att = np.asarray(inputs["b_att"], np.float32)
    idx1 = np.asarray(inputs["idx1"])
    idx2 = np.asarray(inputs["idx2"])
    perms1 = np.asarray(inputs["perms1"])
    perms2 = np.asarray(inputs["perms2"])

    inv2 = np.argsort(perms2, axis=1)
    c = (np.take_along_axis(perms1, inv2, axis=1) == np.arange(K)[None, :]).astype(
        np.float32
    )  # [G, K]
    if (c == 1).all():
        NG = 1
        W_eff = W_eq.mean(axis=0)                                   # [F, COUT]
        coeff = np.ones((E, 1), np.float32)
    else:
        NG = G
        W_eff = np.concatenate([W_eq[g] / G for g in range(G)], axis=1)
        coeff = c[:, idx2].T.copy()                                 # [E, G]
    use_beq = bool(np.any(b_eq != 0.0))
    NO = NG * COUT

    # fold the site tables through the weights (O(nodes), not O(edges))
    A1 = sites1 @ W_eff[0:CIN]              # [B, N1, NO]
    A2 = sites2 @ W_eff[CIN : 2 * CIN]      # [B, K, NO]

    oh1T = (idx1[None, :] == np.arange(N1)[:, None]).astype(np.float32)  # [96, E]
    oh2 = (idx2[:, None] == np.arange(K)[None, :]).astype(np.float32)    # [E, 32]
    oh2T = np.ascontiguousarray(oh2.T)                                   # [32, E]
    bondsT = bonds.transpose(0, 2, 1)                                    # [B, 32, E]

    off = _layouts(NG, use_beq)

    d96_fix = np.zeros((N1, B * NO), np.float32)
    for b in range(B):
        d96_fix[:, b * NO : (b + 1) * NO] = A1[b]

    in_maps = []
    for m in range(NCORES):
        sl = slice(m * ES, (m + 1) * ES)
        d128 = np.zeros((128, off["x128"]), np.float32)
        for ec in range(ECH):
            rows = slice(m * ES + ec * 128, m * ES + (ec + 1) * 128)
            d128[:, off["oh2"] + ec * K : off["oh2"] + (ec + 1) * K] = oh2[rows]
            d128[:, off["coeff"] + ec * NG : off["coeff"] + (ec + 1) * NG] = coeff[rows]
        for b in range(B):
            d128[:, off["wattc"] + b * COUT : off["wattc"] + (b + 1) * COUT] = (
                W_att[:, 0][None, :]
            )
        d128[:, off["batt"]] = b_att[0]
        if use_beq:
            d128[:, off["beq"] : off["beq"] + COUT] = b_eq[None, :]
        d96 = np.empty((N1, off["x96"]), np.float32)
        d96[:, off["oh1T0"] : off["oh1T0"] + 128] = oh1T[:, m * ES : m * ES + 128]
        d96[:, off["A1"] : off["A1"] + B * NO] = d96_fix
        d96[:, off["oh1T1"] :] = oh1T[:, m * ES + 128 : (m + 1) * ES]
        d64 = np.empty((64, off["x64"]), np.float32)
        for b in range(B):
            base = off["combo"] + b * (ES + NO)
            d64[0:CB, base : base + ES] = oh2T[:, sl]
            d64[CB:64, base : base + ES] = bondsT[b][:, sl]
            d64[0:CB, base + ES : base + ES + NO] = A2[b]
            d64[CB:64, base + ES : base + ES + NO] = W_eff[2 * CIN : F]
        in_maps.append({"d128": d128, "d96": d96, "d64": d64})
    return NG, use_beq, in_maps, oh2


def _numpy_fallback(inputs):
    """Exact reference semantics in numpy (only for pathological inputs where
    idx2_oh is not the one-hot of idx2 — never the case for setup_inputs)."""
    sites1 = np.asarray(inputs["sites1"], np.float32)
    sites2 = np.asarray(inputs["sites2"], np.float32)
    bonds = np.asarray(inputs["bonds"], np.float32)
    W_eq = np.asarray(inputs["W_eq"], np.float32)
    b_eq = np.asarray(inputs["b_eq"], np.float32)
    W_att = np.asarray(inputs["W_att"], np.float32)
    b_att = np.asarray(inputs["b_att"], np.float32)
    idx2_oh = np.asarray(inputs["idx2_oh"], np.float32)
    idx1 = np.asarray(inputs["idx1"])
    idx2 = np.asarray(inputs["idx2"])
    perms1 = np.asarray(inputs["perms1"])
    perms2 = np.asarray(inputs["perms2"])
    Gn, Kn = perms1.shape
    inv2 = np.argsort(perms2, axis=1)
    out = np.zeros((B, Kn, COUT), np.float32)
    for b in range(B):
        vec = np.concatenate([sites1[b][idx1], sites2[b][idx2], bonds[b]], axis=1)
        zg = np.stack([vec @ W_eq[g] for g in range(Gn)])        # [G, E, O]
        y = np.zeros((E, COUT, Kn), np.float32)
        for g in range(Gn):
            sel = idx2_oh[:, perms1[g][inv2[g]]]                 # [E, K]
            y += zg[g][:, :, None] * sel[:, None, :]
        y /= Gn
        y = y + b_eq[None, :, None]
        y = np.maximum(y, NEG_SLOPE * y)
        lat = np.einsum("eok,ek->eo", y, idx2_oh)
        att = 1.0 / (1.0 + np.exp(-(lat @ W_att[:, 0] + b_att[0])))
        lat = att[:, None] * lat
        np.add.at(out[b], idx2, lat)
    return out


def _run(inputs, trace=False, **run_kwargs):
    idx2 = np.asarray(inputs["idx2"])
    idx2_oh = np.asarray(inputs["idx2_oh"], np.float32)
    expected_oh = (idx2[:, None] == np.arange(K)[None, :]).astype(np.float32)
    if not np.array_equal(idx2_oh, expected_oh):
        return _numpy_fallback(inputs), None

    NG, use_beq, in_maps, _ = _prepare(inputs)
    nc = _get_program(NG, use_beq)
    res = None
    last_err = None
    for _attempt in range(3):
        try:
            res = run_bass_kernel_spmd(
                nc, in_maps, list(range(NCORES)), trace=trace, **run_kwargs
            )
            break
        except Exception as e:  # transient device/tunnel flakes
            last_err = e
    if res is None:
        raise last_err
    acc = np.zeros((K, B * COUT), np.float32)
    for r in res.results:
        acc += r["out"]
    out = acc.reshape(K, B, COUT).transpose(1, 0, 2)
    return np.ascontiguousarray(out), res


def kernel(**inputs) -> np.ndarray:
    out, _ = _run(inputs)
    return out



# revision 2
# speedup vs baseline: 1.1962x; 1.1962x over previous
"""Trainium2 Bass kernel for nn_MessageUpdatePore (gnn_message_passing).

Algebraic collapse of the reference (valid when idx2_oh == one_hot(idx2) and
perms1 == perms2, which makes the group-averaged equivariant linear fold to
W_eff = mean_g W_eq[g]):
    z[e]  = concat(s1[idx1[e]], s2[idx2[e]], bonds[e]) @ W_eff + b_eq
    lat   = leaky_relu(z); lat *= sigmoid(lat @ W_att + b_att)
    out[b, idx2[e]] += lat                       (scatter-add over edges)

Device-side strategy (edge dim sharded 8 ways, 256 edges/core, bf16):
  - The node-feature gathers fold host-side into a per-edge table
    A12g = (sites1 @ W1)[idx1] + (sites2 @ W2)[idx2]  (O(nodes) matmuls +
    O(E) gather).  On device one matmul per (chunk, batch) computes
        z = [bondsT; A12gT; 1]^T @ [W3; I64; b_eq]
    i.e. the bonds GEMM, the A12g pass-through (identity block), and the
    bias fold into a single 97-deep contraction in PSUM.
  - leaky_relu runs on the otherwise-idle Scalar engine (Lrelu table op,
    PSUM -> SBUF bf16); the attention dot is 4 fused mul+accum DVE ops; one
    Sigmoid instruction covers all 4 (chunk, batch) scores (one act table
    load instead of two); the rescale is split Scalar/DVE.
  - scatter_add is 2 one-hot matmuls (lhsT = oh2 chunk, rhs = both batches).
  - All inputs ride ONE bf16 DRAM tensor DMA'd by the gpsimd SW-DGE ring
    (HWDGE descriptor-gen on sync/scalar measures ~30-60ns/row; gpsimd
    DIRECT2D is ~6ns/row).  Scalar issues no DMAs so its activation-table
    prefetches stay off the critical path.  The [K, B*O] partials are
    summed on the host.
"""

from contextlib import ExitStack

import numpy as np
import ml_dtypes

import concourse.bacc as bacc
import concourse.mybir as mybir
import concourse.tile as tile
from concourse.bass_utils import run_bass_kernel_spmd

B, E, N1, K, CIN, CB, COUT, G = 2, 2048, 96, 32, 64, 32, 64, 4
F = 2 * CIN + CB           # 160
NCORES = 8
ES = E // NCORES           # 256 edges per core
ECH = ES // 128            # 2 edge chunks of 128
NEG_SLOPE = 0.01
f32 = mybir.dt.float32
bf16 = mybir.dt.bfloat16
CROWS = CB + COUT + 1      # 97: bondsT + A12gT + ones/bias row

_programs: dict = {}

# column layout of the single [128, XCOLS] bf16 input tensor
OFF_LHS = 0                        # B blocks of [97, ES] (ec-chunked)
OFF_RHS = B * ES                   # [97, COUT]: W3 | I64 | b_eq
OFF_OH2 = OFF_RHS + COUT           # ECH blocks of [128, K]
OFF_WATT = OFF_OH2 + ECH * K       # [128, COUT] broadcast W_att row
OFF_BATT = OFF_WATT + COUT         # [128, 1]
XCOLS = OFF_BATT + 1               # 705


def _build_program():
    mult = mybir.AluOpType.mult
    nc = bacc.Bacc(
        "TRN2", target_bir_lowering=False, debug=False, num_devices=NCORES
    )
    dAB = nc.dram_tensor("dab", [128, XCOLS], bf16, kind="ExternalInput")
    out_d = nc.dram_tensor("out", [K, B * COUT], f32, kind="ExternalOutput")

    with tile.TileContext(nc) as tc, ExitStack() as ctx:
        const = ctx.enter_context(tc.tile_pool(name="const", bufs=1))
        work = ctx.enter_context(tc.tile_pool(name="work", bufs=2))
        ps_z = ctx.enter_context(tc.tile_pool(name="ps_z", bufs=1, space="PSUM"))
        ps_o = ctx.enter_context(tc.tile_pool(name="ps_o", bufs=1, space="PSUM"))

        tAB = const.tile([128, XCOLS], bf16, tag="tAB", name="tAB")
        nc.gpsimd.dma_start(tAB[:], dAB[:])

        rhs = tAB[0:CROWS, OFF_RHS : OFF_RHS + COUT]
        watt = tAB[:, OFF_WATT : OFF_WATT + COUT]
        batt = tAB[:, OFF_BATT : OFF_BATT + 1]

        # z[(ec,b)] = [bondsT; A12gT; 1]^T @ [W3; I64; b_eq] - one matmul per
        # (chunk, batch) col block, all four sharing one PSUM tile.
        z = ps_z.tile([128, ECH * B * COUT], f32)
        for ec in range(ECH):
            for b in range(B):
                lhsT = tAB[0:CROWS, OFF_LHS + b * ES + ec * 128 : OFF_LHS + b * ES + (ec + 1) * 128]
                c0 = (ec * B + b) * COUT
                nc.tensor.matmul(z[:, c0 : c0 + COUT], lhsT, rhs, start=True, stop=True)

        # leaky_relu on the Scalar engine, whole tile at once, bf16 out
        lat = const.tile([128, ECH * B * COUT], bf16, tag="lat", name="lat")
        nc.scalar.activation(
            lat[:], z[:], mybir.ActivationFunctionType.Lrelu, alpha=NEG_SLOPE
        )

        # attention scores: fused mul + row-accumulate per (chunk, batch)
        s2 = const.tile([128, ECH * B], f32, tag="s2", name="s2")
        for ec in range(ECH):
            for b in range(B):
                i = ec * B + b
                junk = work.tile([128, COUT], bf16, tag="junk", name="junk")
                nc.vector.scalar_tensor_tensor(
                    out=junk[:], in0=lat[:, i * COUT : (i + 1) * COUT], scalar=1.0,
                    in1=watt, op0=mult, op1=mult, accum_out=s2[:, i : i + 1],
                )
        att2 = const.tile([128, ECH * B], f32, tag="att2", name="att2")
        nc.scalar.activation(
            att2[:], s2[:], mybir.ActivationFunctionType.Sigmoid, bias=batt
        )

        # rescale, split across Scalar (Copy w/ per-partition scale) and DVE
        lats = const.tile([128, ECH * B * COUT], bf16, tag="lats", name="lats")
        for ec in range(ECH):
            for b in range(B):
                i = ec * B + b
                sl = slice(i * COUT, (i + 1) * COUT)
                if b == 0:
                    nc.scalar.activation(
                        lats[:, sl], lat[:, sl],
                        mybir.ActivationFunctionType.Copy, scale=att2[:, i : i + 1],
                    )
                else:
                    nc.vector.tensor_scalar_mul(
                        lats[:, sl], lat[:, sl], att2[:, i : i + 1]
                    )

        # scatter_add: one accumulating one-hot matmul per chunk, both batches
        o_ps = ps_o.tile([K, B * COUT], f32)
        for ec in range(ECH):
            oh2c = tAB[:, OFF_OH2 + ec * K : OFF_OH2 + (ec + 1) * K]
            nc.tensor.matmul(
                o_ps[:], oh2c, lats[:, ec * B * COUT : (ec + 1) * B * COUT],
                start=(ec == 0), stop=(ec == ECH - 1),
            )
        o_sb = work.tile([K, B * COUT], f32, tag="osb", name="osb")
        nc.vector.tensor_copy(o_sb[:], o_ps[:])
        nc.gpsimd.dma_start(out_d[:], o_sb[:])

    nc.compile()
    return nc


def _get_program():
    if "v2" not in _programs:
        _programs["v2"] = _build_program()
    return _programs["v2"]


def _prepare(inputs):
    """Host-side preprocessing: weight fold, node-table gather, shard packing."""
    sites1 = np.asarray(inputs["sites1"], np.float32)
    sites2 = np.asarray(inputs["sites2"], np.float32)
    bonds = np.asarray(inputs["bonds"], np.float32)
    W_eq = np.asarray(inputs["W_eq"], np.float32)
    b_eq = np.asarray(inputs["b_eq"], np.float32)
    W_att = np.asarray(inputs["W_att"], np.float32)
    b_att = np.asarray(inputs["b_att"], np.float32)
    idx1 = np.asarray(inputs["idx1"])
    idx2 = np.asarray(inputs["idx2"])

    W_eff = W_eq.mean(axis=0)                       # [F, COUT]
    A1 = sites1 @ W_eff[0:CIN]                      # [B, N1, COUT]
    A2 = sites2 @ W_eff[CIN : 2 * CIN]              # [B, K, COUT]
    A12g = A1[:, idx1] + A2[:, idx2]                # [B, E, COUT]
    W3 = W_eff[2 * CIN : F]                         # [CB, COUT]
    oh2 = (idx2[:, None] == np.arange(K)[None, :])  # [E, K]

    in_maps = []
    for m in range(NCORES):
        sl = slice(m * ES, (m + 1) * ES)
        d = np.zeros((128, XCOLS), ml_dtypes.bfloat16)
        for b in range(B):
            blk = slice(OFF_LHS + b * ES, OFF_LHS + (b + 1) * ES)
            d[0:CB, blk] = bonds[b, sl].T
            d[CB : CB + COUT, blk] = A12g[b, sl].T
            d[CB + COUT, blk] = 1.0
        d[0:CB, OFF_RHS : OFF_RHS + COUT] = W3
        d[CB : CB + COUT, OFF_RHS : OFF_RHS + COUT] = np.eye(COUT)
        d[CB + COUT, OFF_RHS : OFF_RHS + COUT] = b_eq
        for ec in range(ECH):
            rows = slice(m * ES + ec * 128, m * ES + (ec + 1) * 128)
            d[:, OFF_OH2 + ec * K : OFF_OH2 + (ec + 1) * K] = oh2[rows]
        d[:, OFF_WATT : OFF_WATT + COUT] = W_att[:, 0][None, :]
        d[:, OFF_BATT] = b_att[0]
        in_maps.append({"dab": d})
    return in_maps


def _numpy_fallback(inputs):
    """Exact reference semantics in numpy (only for pathological inputs where
    idx2_oh is not the one-hot of idx2 or the perms do not fold — never the
    case for setup_inputs)."""
    sites1 = np.asarray(inputs["sites1"], np.float32)
    sites2 = np.asarray(inputs["sites2"], np.float32)
    bonds = np.asarray(inputs["bonds"], np.float32)
    W_eq = np.asarray(inputs["W_eq"], np.float32)
    b_eq = np.asarray(inputs["b_eq"], np.float32)
    W_att = np.asarray(inputs["W_att"], np.float32)
    b_att = np.asarray(inputs["b_att"], np.float32)
    idx2_oh = np.asarray(inputs["idx2_oh"], np.float32)
    idx1 = np.asarray(inputs["idx1"])
    idx2 = np.asarray(inputs["idx2"])
    perms1 = np.asarray(inputs["perms1"])
    perms2 = np.asarray(inputs["perms2"])
    Gn, Kn = perms1.shape
    inv2 = np.argsort(perms2, axis=1)
    out = np.zeros((B, Kn, COUT), np.float32)
    for b in range(B):
        vec = np.concatenate([sites1[b][idx1], sites2[b][idx2], bonds[b]], axis=1)
        zg = np.stack([vec @ W_eq[g] for g in range(Gn)])        # [G, E, O]
        y = np.zeros((E, COUT, Kn), np.float32)
        for g in range(Gn):
            sel = idx2_oh[:, perms1[g][inv2[g]]]                 # [E, K]
            y += zg[g][:, :, None] * sel[:, None, :]
        y /= Gn
        y = y + b_eq[None, :, None]
        y = np.maximum(y, NEG_SLOPE * y)
        lat = np.einsum("eok,ek->eo", y, idx2_oh)
        att = 1.0 / (1.0 + np.exp(-(lat @ W_att[:, 0] + b_att[0])))
        lat = att[:, None] * lat
        np.add.at(out[b], idx2, lat)
    return out


def _run(inputs, trace=False, **run_kwargs):
    idx2 = np.asarray(inputs["idx2"])
    idx2_oh = np.asarray(inputs["idx2_oh"], np.float32)
    expected_oh = (idx2[:, None] == np.arange(K)[None, :]).astype(np.float32)
    perms1 = np.asarray(inputs["perms1"])
    perms2 = np.asarray(inputs["perms2"])
    inv2 = np.argsort(perms2, axis=1)
    folds = (np.take_along_axis(perms1, inv2, axis=1) == np.arange(K)[None, :]).all()
    if not np.array_equal(idx2_oh, expected_oh) or not folds:
        return _numpy_fallback(inputs), None

    in_maps = _prepare(inputs)
    nc = _get_program()
    res = None
    last_err = None
    for _attempt in range(3):
        try:
            res = run_bass_kernel_spmd(
                nc, in_maps, list(range(NCORES)), trace=trace, **run_kwargs
            )
            break
        except Exception as e:  # transient device/tunnel flakes
            last_err = e
    if res is None:
        raise last_err
    acc = np.zeros((K, B * COUT), np.float32)
    for r in res.results:
        acc += r["out"]
    out = acc.reshape(K, B, COUT).transpose(1, 0, 2)
    return np.ascontiguousarray(out), res


def kernel(**inputs) -> np.ndarray:
    out, _ = _run(inputs)
    return out
